# revision 26
# baseline (speedup 1.0000x reference)
# Trainium2 Bass kernel for dynamic-routing capsule layer (nn_Capsule).
#
# Math (per batch b):
#   u_hat[n,i,j] = sum_d u[n,d] W[d, i*16+j]
#   b=0; for it in 0..2:
#     c = softmax(b, axis=i)
#     o[i,j] = sum_n c[i,n] u_hat[n,i,j]
#     if it<2: o' = l2norm(o); b[i,n] = sum_j o'[i,j] u_hat[n,i,j]
#   out = squash(o)
#
# Restructuring (u_hat [B,N,512] never materialized); all PE operands bf16;
# the 4 local batches are stacked on partitions/free dims so wide-vector ops
# run once per iteration instead of once per batch:
#   sT[d, bb, i] = sum_n u[n,d] c[bb,i,n]      (PE: u stationary, cT moving F=32)
#   O[bb*32+i,f] = sum_d sT[d,bb,i] W[d,f]     (PE: W moving)
#   o'[i,j] via mask+rsqrt(nrm) fold           (DVE Newton rsqrt, no ACT tables)
#   E[q,f]  = o'_flat[f] broadcast             (PE rank-1 matmul)
#   vT[d,bb,i] = sum_j W[d,16i+j] E[d,16i+j]   (DVE mul + Pool group-reduce)
#   bT[n, bb*32+i] = sum_d uT[d,n] vT[d,bb,i]  (PE: uT stationary, vT moving F=32)
#   softmax over i directly in token-partition layout -> cT for next iter
# No e-transposes; uT via PE transposes during the u DMA stream.
# ACT runs only Exp/Square/Copy (one act-table set -> single table load).
# Iteration 0 uses uniform c (softmax of zeros): s0 is rank-1 per batch, so
# mm1-it0 collapses to F=1 matmuls and o'_0 comes from a [1,512] row.
#
# Sharding: data-parallel over batch B=32 across 8 cores (4 batches/core),
# W replicated. No collectives.

import numpy as np

N_CORES = 8
B, N, D = 32, 4096, 256
I_CAPS, J_DIM = 32, 16
ROUTINGS = 3
EPS = 1e-7
L2_EPS = 1e-12

NB4 = 4          # batches per core
NR_ITERS = 1     # Newton iterations for rsqrt (seed err 3.4% -> ~0.2%)
N_DMAT = 2       # batches whose uT comes from DmaTranspose (the late ones)


def build_nc(b_loc=B // N_CORES, n=N, d=D, enable_asserts=False, reps=1):
    from contextlib import ExitStack

    import concourse.bass as bass  # noqa: F401
    import concourse.tile as tile
    from concourse import bacc, mybir
    from concourse.masks import make_identity
    import bass_rust

    assert b_loc == NB4

    def chain(insts):
        # same-engine ordering edges: keeps a psum accumulation mega-group's
        # start=True member first and preserves pack order without semaphores
        for a, b2 in zip(insts[1:], insts[:-1]):
            bass_rust.add_dep_helper(a.ins, b2.ins, sync=False,
                                     reason="pack order")

    f32 = mybir.dt.float32
    bf16 = mybir.dt.bfloat16
    i32 = mybir.dt.int32
    AX = mybir.AxisListType
    OP = mybir.AluOpType
    ACTF = mybir.ActivationFunctionType

    NC = n // 128        # 32 token chunks of 128
    DC = d // 128        # 2 d-halves
    QN = NC // 4         # tokens chunks per DMA quarter (8)
    IJ = I_CAPS * J_DIM  # 512

    nc = bacc.Bacc("TRN2", target_bir_lowering=False, debug=False,
                   enable_asserts=enable_asserts)
    u_dram = nc.dram_tensor("u", [b_loc, n, d], f32, kind="ExternalInput").ap()
    w_dram = nc.dram_tensor("w", [1, d, IJ], f32, kind="ExternalInput").ap()
    out_dram = nc.dram_tensor("out", [b_loc, I_CAPS, J_DIM], f32,
                              kind="ExternalOutput").ap()

    with tile.TileContext(nc) as tc, ExitStack() as ctx:
        const_pool = ctx.enter_context(tc.tile_pool(name="const", bufs=1))
        u_pool = ctx.enter_context(tc.tile_pool(name="u", bufs=NB4))
        uT_pool = ctx.enter_context(tc.tile_pool(name="uT", bufs=NB4))
        cT_pool = ctx.enter_context(tc.tile_pool(name="cT", bufs=2))
        e_pool = ctx.enter_context(tc.tile_pool(name="e", bufs=2))
        med = ctx.enter_context(tc.tile_pool(name="med", bufs=2))
        tiny = ctx.enter_context(tc.tile_pool(name="tiny", bufs=2))
        psum = ctx.enter_context(tc.tile_pool(name="ps", bufs=1, space="PSUM"))

        # ---------------- constants ----------------
        ident = const_pool.tile([128, 128], f32, name="ident")
        make_identity(nc, ident[:])
        ident_b = const_pool.tile([128, 128], bf16, name="ident_b")
        nc.vector.tensor_copy(ident_b[:], ident[:])

        ones_b = const_pool.tile([128, 128], bf16, name="ones_b")
        scratchf = med.tile([128, 128], f32, tag="scratchf", name="scratchf")
        nc.gpsimd.memset(scratchf[:], 1.0)
        nc.vector.tensor_copy(ones_b[:], scratchf[:])

        # uniform routing weight column for iteration 0
        c0 = const_pool.tile([128, 1], bf16, name="c0")
        nc.gpsimd.memset(c0[:], 1.0 / I_CAPS)

        # rsqrt magic constant
        k5f = const_pool.tile([128, 1], i32, name="k5f")
        nc.gpsimd.memset(k5f[:], 0x5F3759DF)

        # MASK[32*bb + i, f] = 1 if f // 16 == i else 0   ([128, 512] bf16)
        maskf = med.tile([128, IJ], f32, tag="maskf", name="maskf")
        nc.gpsimd.memset(maskf[:], 0.0)
        for bb in range(NB4):
            mslice = maskf[bb * I_CAPS:(bb + 1) * I_CAPS, :]
            nc.gpsimd.affine_select(
                out=mslice, in_=mslice, compare_op=OP.is_gt, fill=1.0,
                base=-(J_DIM - 1), pattern=[[1, IJ]], channel_multiplier=-J_DIM)
            nc.gpsimd.affine_select(
                out=mslice, in_=mslice, compare_op=OP.is_ge, fill=0.0,
                base=0, pattern=[[1, IJ]], channel_multiplier=-J_DIM)
        mask_all = const_pool.tile([128, IJ], bf16, name="mask_all")
        nc.vector.tensor_copy(mask_all[:], maskf[:])

        # W natural: w_sb[q, e, f] = W[128e+q, f]; bf16 for the routing path,
        # f32r for the final-iteration O (full precision on the output path)
        w_sb = const_pool.tile([128, DC, IJ], bf16, name="w_sb")
        nc.gpsimd.dma_start(w_sb[:], w_dram[0].rearrange("(e q) f -> q e f", q=128))
        w_sbr = w_sb  # f32r stationary cannot target psum partition!=0; keep bf16

        def nr_rsqrt(x_ap, width, label):
            """y ~ 1/sqrt(x) on DVE only (no ACT tables). x_ap: [128, width] f32."""
            shr = tiny.tile([128, width], i32, tag="nr_shr", name=f"shr_{label}")
            nc.vector.tensor_scalar(shr[:], x_ap.bitcast(i32), 1, None,
                                    op0=OP.logical_shift_right)
            seed = tiny.tile([128, width], f32, tag="nr_seed", name=f"seed_{label}")
            k_ap = (k5f[:, 0:1] if width == 1
                    else k5f[:, 0:1].broadcast_to([128, width]))
            nc.vector.tensor_tensor(seed[:].bitcast(i32), k_ap, shr[:],
                                    op=OP.subtract)
            y = seed
            for it in range(NR_ITERS):
                t = tiny.tile([128, width], f32, tag="nr_t", name=f"nrt_{label}_{it}")
                nc.vector.tensor_mul(t[:], y[:], y[:])
                nc.vector.tensor_mul(t[:], t[:], x_ap)
                h = tiny.tile([128, width], f32, tag="nr_h", name=f"nrh_{label}_{it}")
                nc.vector.tensor_scalar(h[:], t[:], -0.5, 1.5,
                                        op0=OP.mult, op1=OP.add)
                y2 = tiny.tile([128, width], f32, tag="nr_y", name=f"nry_{label}_{it}")
                nc.vector.tensor_mul(y2[:], y[:], h[:])
                y = y2
            return y

        for rep in range(reps):
            _body(nc, tc, mybir, chain, nr_rsqrt, rep,
                  n, d, NC, DC, QN, IJ, f32, bf16, i32, AX, OP, ACTF,
                  u_dram, out_dram,
                  u_pool, uT_pool, cT_pool, e_pool, med, tiny, psum,
                  ident_b, ones_b, c0, mask_all, w_sb, w_sbr)

    nc.compile()
    return nc


def _body(nc, tc, mybir, chain, nr_rsqrt, rep,
          n, d, NC, DC, QN, IJ, f32, bf16, i32, AX, OP, ACTF,
          u_dram, out_dram,
          u_pool, uT_pool, cT_pool, e_pool, med, tiny, psum,
          ident_b, ones_b, c0, mask_all, w_sb, w_sbr):
    # ---------------- load + transpose + iter-0 partial mm1 ----------------
    # u_t[bb][p, c, dd] = u[bb, 32p + c, dd]  (token nn = 32p + c)
    # uT_t[bb][q, c, e, p] = u[bb, 32p + c, 128e + q]
    # Early batches transpose on the PE (DVE/ACT evac); the last N_DMAT
    # batches use the DMA xbar transpose (idle DMA capacity, no evac).
    u_ts, uT_ts = [], []
    sT0_ps = psum.tile([128, 512], f32, tag="sT", bufs=1, name=f"sT0_{rep}")
    mm1_0 = []
    evac_flip = 0
    HN = NC // 2  # token chunks per DMA half
    for bb in range(NB4):
        u_t = u_pool.tile([128, NC, d], bf16, tag="u", name=f"u_{rep}_{bb}")
        uT_t = uT_pool.tile([128, NC, DC, 128], bf16, tag="uT",
                            name=f"uT_{rep}_{bb}")
        u_ts.append(u_t)
        uT_ts.append(uT_t)
        use_dmat = bb >= NB4 - N_DMAT
        for h in range(2):
            nc.gpsimd.dma_start(
                u_t[:, h * HN:(h + 1) * HN, :],
                u_dram[bb].rearrange("(p c) dd -> p c dd",
                                     c=NC)[:, h * HN:(h + 1) * HN, :])
            if use_dmat:
                nc.sync.dma_start_transpose(
                    uT_t[:, h * HN:(h + 1) * HN, :, :],
                    u_t[:, h * HN:(h + 1) * HN, :])
            else:
                for q in range(h * 2, h * 2 + 2):
                    for e in range(DC):
                        # transpose group: 8 tiles -> one psum bank (bf16)
                        tr_ps = psum.tile([128, QN, 128], bf16, tag="tr",
                                          bufs=2, name=f"tr_{rep}_{bb}_{q}_{e}")
                        pack = []
                        for k in range(QN):
                            c = q * QN + k
                            pack.append(nc.tensor.matmul(
                                tr_ps[:, k, :],
                                u_t[:, c, e * 128:(e + 1) * 128],
                                ident_b[:],
                                is_transpose=True,
                                start=(k == 0), stop=(k == QN - 1)))
                        chain(pack)
                        dst = uT_t[:, q * QN:(q + 1) * QN, e, :]
                        if evac_flip % 2 == 0:
                            nc.vector.tensor_copy(dst, tr_ps[:])
                        else:
                            nc.scalar.copy(dst, tr_ps[:])
                        evac_flip += 1
            # iter-0 mm1 (uniform c): sT0[q, 4e+bb] += sum_p u_t[p,c,q+128e]/32
            n_mm1_0 = NB4 * 2 * DC * HN
            for e in range(DC):
                col = 4 * e + bb
                for k in range(HN):
                    c = h * HN + k
                    mm1_0.append(nc.tensor.matmul(
                        sT0_ps[:, col:col + 1],
                        u_t[:, c, e * 128:(e + 1) * 128],
                        c0[:],
                        start=(len(mm1_0) == 0),
                        stop=(len(mm1_0) == n_mm1_0 - 1),
                        skip_group_check=True))
    chain(mm1_0)

    sT0_sb = tiny.tile([128, 2 * NB4], bf16, tag="sT0", name=f"sT0sb_{rep}")
    nc.vector.tensor_copy(sT0_sb[:], sT0_ps[:, 0:2 * NB4])

    cT_all = None
    sT_ps = None
    for it in range(ROUTINGS):
        # ---------------- O = S @ W ----------------
        if it == 0:
            # rank-1 per batch: O0 row 32bb holds o0_flat[f] = s0 . W[:, f]
            o_ps = psum.tile([128, IJ], f32, tag="O", bufs=1,
                             name=f"O_{rep}_{it}")
            pack = []
            for bb in range(NB4):
                for e in range(DC):
                    col = 4 * e + bb
                    pack.append(nc.tensor.matmul(
                        o_ps[32 * bb:32 * bb + 1, :],
                        sT0_sb[:, col:col + 1],
                        w_sb[:, e, :],
                        start=(e == 0), stop=(e == DC - 1),
                        tile_position=(0, 32 * bb),
                        skip_group_check=True))
            chain(pack)
            # nrm0[32bb, i] = sum_j O0[32bb, 16i+j]^2  (junk on other rows)
            sq = med.tile([128, IJ], bf16, tag="sq0", name=f"sq0_{rep}")
            nc.scalar.activation(sq[:], o_ps[:], ACTF.Square)
            nrm0 = tiny.tile([128, I_CAPS], f32, tag="nrm0", name=f"nrm0_{rep}")
            nc.vector.tensor_reduce(
                nrm0[:], sq[:].rearrange("p (i j) -> p i j", j=J_DIM),
                axis=AX.X, op=OP.add)
            rr0 = nr_rsqrt(nrm0[:], I_CAPS, f"rr0_{rep}")
            # om0[32bb, 16i+j] = o'_flat (normalized), bf16
            om = med.tile([128, IJ], bf16, tag="om", name=f"om_{rep}_{it}")
            nc.vector.tensor_mul(
                om[:].rearrange("p (i j) -> p i j", j=J_DIM),
                o_ps[:].rearrange("p (i j) -> p i j", j=J_DIM),
                rr0[:].unsqueeze(2).broadcast_to([128, I_CAPS, J_DIM]))
        else:
            # evac sT (cols: 32*(4e+bb)+i) and run the full S@W
            sT_sb = tiny.tile([128, DC, NB4, I_CAPS], bf16, tag="sT_sb",
                              name=f"sTsb_{rep}_{it}")
            nc.vector.tensor_copy(sT_sb[:], sT_ps[:, 0:DC * NB4 * I_CAPS])
            o_ps = psum.tile([128, IJ], f32, tag="O", bufs=1,
                             name=f"O_{rep}_{it}")
            pack = []
            w_o = w_sb if it < ROUTINGS - 1 else w_sbr
            for bb in range(NB4):
                for e in range(DC):
                    pack.append(nc.tensor.matmul(
                        o_ps[32 * bb:32 * (bb + 1), :],
                        sT_sb[:, e, bb, :],
                        w_o[:, e, :],
                        start=(e == 0), stop=(e == DC - 1),
                        tile_position=(0, 32 * bb),
                        skip_group_check=True))
            chain(pack)
            om_dt = bf16 if it < ROUTINGS - 1 else f32
            om = med.tile([128, IJ], om_dt, tag=f"om{it}", name=f"om_{rep}_{it}")
            nc.vector.tensor_mul(om[:], o_ps[:], mask_all[:])

        if it < ROUTINGS - 1:
            if it > 0:
                # nrm[p,1] = sum_f om^2 (ACT Square + accumulator; table set 0)
                sqa = med.tile([128, IJ], bf16, tag="sqa", name=f"sqa_{rep}_{it}")
                nrm = tiny.tile([128, 1], f32, tag="nrm", name=f"nrm_{rep}_{it}")
                nc.scalar.activation(sqa[:], om[:], ACTF.Square, accum_out=nrm[:])
                rr = nr_rsqrt(nrm[:], 1, f"rr_{rep}_{it}")
                rrb = tiny.tile([128, 128], bf16, tag="rrb", name=f"rrb_{rep}_{it}")
                nc.vector.tensor_scalar_mul(rrb[:], ones_b[:], rr[:, 0:1])

            # ---------------- E broadcast + vT ----------------
            vT_sb = tiny.tile([128, DC, NB4, I_CAPS], bf16, tag="vT",
                              name=f"vT_{rep}_{it}")
            for bb in range(NB4):
                e_ps = psum.tile([128, IJ], f32, tag="E", bufs=2,
                                 name=f"E_{rep}_{it}_{bb}")
                if it == 0:
                    nc.tensor.matmul(
                        e_ps[:],
                        ones_b[32 * bb:32 * bb + 1, :],
                        om[32 * bb:32 * bb + 1, :],
                        start=True, stop=True,
                        tile_position=(32 * bb, 0))
                else:
                    nc.tensor.matmul(
                        e_ps[:],
                        rrb[32 * bb:32 * (bb + 1), :],
                        om[32 * bb:32 * (bb + 1), :],
                        start=True, stop=True,
                        tile_position=(32 * bb, 0))
                # evac E on ACT so the W*E muls run at DVE 2x from SBUF
                e_sbE = med.tile([128, IJ], bf16, tag="Esb",
                                 name=f"Esb_{rep}_{it}_{bb}")
                nc.scalar.copy(e_sbE[:], e_ps[:])
                for e in range(DC):
                    wtmp = med.tile([128, IJ], bf16, tag="wtmp",
                                    name=f"wtmp_{rep}_{it}_{bb}_{e}")
                    nc.vector.tensor_mul(wtmp[:], w_sb[:, e, :], e_sbE[:])
                    vT_f = tiny.tile([128, I_CAPS], f32, tag="vT_f",
                                     name=f"vTf_{rep}_{it}_{bb}_{e}")
                    nc.vector.tensor_reduce(
                        vT_f[:],
                        wtmp[:].rearrange("q (i j) -> q i j", j=J_DIM),
                        axis=AX.X, op=OP.add)
                    nc.vector.tensor_copy(vT_sb[:, e, bb, :], vT_f[:])

            # ---------------- mm2 (bT form) + exp ----------------
            # e/cT memory layout [128, bb, i, c] (c innermost) so softmax's
            # mul runs at DVE 2x (packed last dim on every operand)
            e_all = e_pool.tile([128, NB4, I_CAPS, NC], bf16, tag="e",
                                name=f"e_{rep}_{it}")
            GC = 4  # c-chunks per psum bank
            for g in range(NC // GC):
                bT_ps = psum.tile([128, GC, 128], f32, tag="bT", bufs=2,
                                  name=f"bT_{rep}_{it}_{g}")
                pack = []
                for cc in range(GC):
                    c = g * GC + cc
                    for bb in range(NB4):
                        for e in range(DC):
                            pack.append(nc.tensor.matmul(
                                bT_ps[:, cc, 32 * bb:32 * (bb + 1)],
                                uT_ts[bb][:, c, e, :],
                                vT_sb[:, e, bb, :],
                                start=(len(pack) == 0),
                                stop=(len(pack) == GC * NB4 * DC - 1),
                                skip_group_check=True))
                chain(pack)
                nc.scalar.activation(
                    e_all[:, :, :, g * GC:(g + 1) * GC].rearrange(
                        "p bb i c -> p c bb i"),
                    bT_ps[:], ACTF.Exp)

            # ---------------- softmax over i + next-iter mm1 ----------------
            cT_all = cT_pool.tile([128, NB4, I_CAPS, NC], bf16, tag="cT",
                                  name=f"cT_{rep}_{it + 1}")
            z = tiny.tile([128, NB4, NC], f32, tag="z", name=f"z_{rep}_{it}")
            r = tiny.tile([128, NB4, NC], bf16, tag="r", name=f"r_{rep}_{it}")
            sT_ps = psum.tile([128, 512], f32, tag="sT", bufs=1,
                              name=f"sT_{rep}_{it + 1}")
            mm1 = []
            for g in range(NC // GC):
                c0g, c1g = g * GC, (g + 1) * GC
                nc.vector.tensor_reduce(
                    z[:, :, c0g:c1g],
                    e_all[:, :, :, c0g:c1g].rearrange("p bb i c -> p bb c i"),
                    axis=AX.X, op=OP.add)
                with nc.allow_low_precision(reason="bf16 softmax recip"):
                    nc.vector.reciprocal(r[:, :, c0g:c1g], z[:, :, c0g:c1g])
                nc.vector.tensor_mul(
                    cT_all[:, :, :, c0g:c1g], e_all[:, :, :, c0g:c1g],
                    r[:, :, c0g:c1g].unsqueeze(2).broadcast_to(
                        [128, NB4, I_CAPS, GC]))
                for cc in range(GC):
                    c = c0g + cc
                    for bb in range(NB4):
                        for e in range(DC):
                            col = 32 * (4 * e + bb)
                            mm1.append(nc.tensor.matmul(
                                sT_ps[:, col:col + I_CAPS],
                                u_ts[bb][:, c, e * 128:(e + 1) * 128],
                                cT_all[:, bb, :, c],
                                start=(len(mm1) == 0),
                                stop=(len(mm1) == NC * NB4 * DC - 1),
                                skip_group_check=True))
            chain(mm1)
        else:
            # ---------------- squash + output ----------------
            o_all = tiny.tile([128, J_DIM], f32, tag="o", name=f"o_{rep}")
            nc.vector.tensor_reduce(
                o_all[:], om[:].rearrange("p (i j) -> p j i", j=J_DIM),
                axis=AX.X, op=OP.add)
            sqa = med.tile([128, IJ], bf16, tag="sqa", name=f"sqa_{rep}_{it}")
            nrm = tiny.tile([128, 1], f32, tag="nrm", name=f"nrm_{rep}_{it}")
            nc.scalar.activation(sqa[:], om[:], ACTF.Square, accum_out=nrm[:])
            s2 = tiny.tile([128, 1], f32, tag="s2", name=f"s2_{rep}")
            nc.vector.tensor_scalar_add(s2[:], nrm[:], EPS)
            rr2 = nr_rsqrt(s2[:], 1, f"sq_{rep}")
            sqr = tiny.tile([128, 1], f32, tag="sqr", name=f"sqr_{rep}")
            nc.vector.tensor_mul(sqr[:], s2[:], rr2[:])   # sqrt(s2)
            den = tiny.tile([128, 1], f32, tag="den", name=f"den_{rep}")
            nc.vector.tensor_scalar_add(den[:], s2[:], 0.5)
            rden = tiny.tile([128, 1], f32, tag="rden", name=f"rden_{rep}")
            nc.vector.reciprocal(rden[:], den[:])
            scl = tiny.tile([128, 1], f32, tag="scl", name=f"scl_{rep}")
            nc.vector.tensor_mul(scl[:], sqr[:], rden[:])
            o_out = tiny.tile([128, J_DIM], f32, tag="oout", name=f"oout_{rep}")
            nc.vector.tensor_scalar_mul(o_out[:], o_all[:], scl[:, 0:1])
            nc.sync.dma_start(
                out_dram.rearrange("b i j -> (b i) j"), o_out[:])


_NC_CACHE = {}


def _get_nc():
    if "nc" not in _NC_CACHE:
        _NC_CACHE["nc"] = build_nc()
    return _NC_CACHE["nc"]


def kernel(u_vecs: np.ndarray, W: np.ndarray) -> np.ndarray:
    from concourse.bass_utils import run_bass_kernel_spmd

    u_vecs = np.ascontiguousarray(u_vecs, dtype=np.float32)
    W = np.ascontiguousarray(W, dtype=np.float32)
    b_loc = B // N_CORES
    nc = _get_nc()
    in_maps = [
        {"u": u_vecs[i * b_loc:(i + 1) * b_loc], "w": W}
        for i in range(N_CORES)
    ]
    res = run_bass_kernel_spmd(nc, in_maps, core_ids=list(range(N_CORES)))
    return np.concatenate([r["out"] for r in res.results], axis=0)
